# revision 21
# baseline (speedup 1.0000x reference)
"""DTW loss kernel for Trainium2 (Bass).

Computes sqrt(DTW^2(source, target)) for source, target of shape (2048,)
via the standard DP:
    D[i,j] = (s_i - t_j)^2 + min(D[i-1,j], D[i,j-1], D[i-1,j-1])

Device mapping (one NeuronCore; the problem is a single (source,target)
pair, so all 8 cores run the same program replicated and core 0's output
is used):

- 128 column-chunks of 16 columns each; partition p owns columns [16p,16p+16).
- Wavefront: at step t partition p computes DP row r = t - 2*p (2 steps of
  slack per chunk so the boundary machinery stays off the critical path).
- One DP row-chunk = ONE vector-engine tensor_tensor_scan instruction:
  state = min(d0, state) + d1 over 32 interleaved slots (2 per cell):
    slot 2j   : d0 = D[r-1, j]   (up),      d1 = 0
    slot 2j+1 : d0 = D[r-1, j-1] (upleft),  d1 = c[r, j]
  so after slot 2j+1: state = c + min(up, upleft, left-carry)  == D[r, j].
  d0 is a 3-D access pattern (positions 2+2j-2i) over the previous row strip.
- Cross-chunk boundary (D[r, 16p-1]): PE matmul with a shifted-identity matrix
  moves each chunk's last column value to partition p+1 (PSUM), then the
  scalar engine copies it into the next strip's halo slot, adding a
  per-partition bias [1e30, 0, ...] to restore partition 0's boundary = INF.
- Costs c[r,j] are produced in bulk by the scalar engine (Square activation
  with per-partition bias -t_j) into a ring buffer from a diagonally-shifted
  copy of source.

Host<->device traffic is kept minimal: only source (8KB) and -target (8KB)
are shipped per core; the diagonally-shifted source copy, the shift matrix
and the boundary bias are constructed on device (memsets + two DMAs, one of
them a stride-0 partition broadcast with a skewed output access pattern).

Dispatch: run_bass_kernel_spmd re-traces and re-compiles a fresh
jax.jit(shard_map(...)) on every call (~0.9s each under axon). The first
kernel() call goes through it once; subsequent calls reuse a process-cached
jit of the identical _bass_exec_p computation, so a steady-state call costs
one axon round trip.
"""

import os
import sys

for _p in ("/opt/trn_rl_repo", "/root/.axon_site/_ro/trn_rl_repo"):
    if os.path.isdir(_p) and _p not in sys.path:
        sys.path.insert(0, _p)

import numpy as np

import concourse.bass as bass
import concourse.bacc as bacc
import concourse.mybir as mybir
import concourse.tile as tile
from concourse.bass_utils import run_bass_kernel_spmd

F32 = mybir.dt.float32

N = 2048            # sequence length (both source and target)
P = 128             # partitions / column chunks
CW = N // P         # columns per chunk
SW = 2 * CW + 2     # strip width: [halo | 2*CW scan slots | pad]
SLACK = 4           # wavefront steps of slack per chunk (= number of strips)
T = N + SLACK * (P - 1)   # 2302 total wavefront steps
M = T + 2                 # sdiag columns (padded)
RANGE = 256               # ring refill granularity (steps)
NRANGE = (T + RANGE - 1) // RANGE
RING = 3 * RANGE          # ring capacity in row-slots
INF = np.float32(1e30)
PAD = np.float32(1e15)    # sdiag pad; squares to 1e30
NCORES = 8

_cache = {}


def _build(debug_dump=False):
    nc = bacc.Bacc("TRN2", target_bir_lowering=False, debug=False)

    src = nc.dram_tensor("src", [1, N], F32, kind="ExternalInput")
    negt = nc.dram_tensor("negt", [P, CW], F32, kind="ExternalInput")
    res = nc.dram_tensor("res", [1, 1], F32, kind="ExternalOutput")
    if debug_dump:
        dbg_sdiag = nc.dram_tensor("dbg_sdiag", [P, M], F32, kind="ExternalOutput")
        dbg_shift = nc.dram_tensor("dbg_shift", [P, P], F32, kind="ExternalOutput")
        dbg_bias = nc.dram_tensor("dbg_bias", [P, 1], F32, kind="ExternalOutput")

    with tile.TileContext(nc) as tc:
        with (
            tc.tile_pool(name="sb", bufs=1) as pool,
            tc.tile_pool(name="ps", bufs=8, space="PSUM") as psp,
        ):
            t_sdiag = pool.tile([P, M], F32)
            t_negt = pool.tile([P, CW], F32)
            t_shift = pool.tile([P, P], F32)
            t_bias = pool.tile([P, 1], F32)
            t_ring = pool.tile([P, RING * 2 * CW], F32)
            t_tmp = pool.tile([P, RANGE], F32)
            strips = [
                pool.tile([P, SW], F32, name=f"strip{i}") for i in range(SLACK)
            ]

            nc.sync.dma_start(t_negt[:], negt[:])

            # sdiag[p, SLACK*p : SLACK*p + N] = source, PAD elsewhere.
            # Built by log-doubling SBUF->SBUF copies: seed row 0, then each
            # round copies partitions [0, 2^k) to [2^k, 2^{k+1}) at a uniform
            # free offset +SLACK*2^k. Every AP uses the exact partition pitch
            # and constant offsets (skewed partition strides mis-lower in the
            # HW DGE, which restarts accumulation at 4-partition granules).
            nc.vector.memset(t_sdiag[:], float(PAD))
            sstr = int(t_sdiag.ap[0][0])
            nc.sync.dma_start(t_sdiag[0:1, 0:N], src[:])
            for k in range(7):
                blk = 1 << k
                w = N + SLACK * (blk - 1)
                sd_in = bass.AP(t_sdiag.tensor, t_sdiag.offset, [[sstr, blk], [1, w]])
                sd_out = bass.AP(
                    t_sdiag.tensor,
                    t_sdiag.offset + blk * sstr + SLACK * blk,
                    [[sstr, blk], [1, w]],
                )
                nc.sync.dma_start(sd_out, sd_in)

            # shift matrix: ones on the (p-1, p) diagonal, via affine_select
            # (iota = q - p - 1 == 0 keeps the broadcast 1.0, else fill 0).
            # A strided 4-byte SBUF->SBUF DMA scatter writes garbage on HW,
            # and engines can't address single partitions at arbitrary bases.
            nc.gpsimd.memset(t_shift[:], 1.0)
            nc.gpsimd.affine_select(
                t_shift[:],
                t_shift[:],
                pattern=[[1, P]],
                compare_op=mybir.AluOpType.is_equal,
                fill=0.0,
                base=-1,
                channel_multiplier=-1,
            )

            # boundary bias: [INF, 0, 0, ...]
            nc.vector.memset(t_bias[:], 0.0)
            nc.vector.memset(t_bias[0:1, 0:1], float(INF))

            # zeros in the even (d1) slots persist for the whole run
            nc.gpsimd.memset(t_ring[:], 0.0)
            for s in strips:
                nc.vector.memset(s[:], float(INF))
            # corner DTW[0,0] = 0 for the virtual row read by scan(0)
            # (scan(0)'s prev strip is strips[SLACK-1])
            nc.vector.memset(strips[SLACK - 1][0:1, 0:1], 0.0)

            eng = nc.vector

            def refill(g):
                # Costs are produced on the Pool engine (add then square) so
                # the scalar engine only runs the per-step boundary copy and
                # refill bursts never queue in front of it.
                if g >= NRANGE:
                    return
                m0 = g * RANGE
                cnt = min(RANGE, M - m0)
                base = (m0 % RING) * 2 * CW
                rstr = int(t_ring.ap[0][0])
                for j in range(CW):
                    out_ap = bass.AP(
                        t_ring.tensor,
                        t_ring.offset + base + 2 * j + 1,
                        [[rstr, P], [2 * CW, cnt]],
                    )
                    nc.gpsimd.tensor_scalar(
                        t_tmp[:, 0:cnt],
                        t_sdiag[:, m0 : m0 + cnt],
                        t_negt[:, j : j + 1],
                        None,
                        mybir.AluOpType.add,
                    )
                    nc.gpsimd.tensor_mul(out_ap, t_tmp[:, 0:cnt], t_tmp[:, 0:cnt])

            refill(0)
            refill(1)
            refill(2)

            pstr = int(strips[0].ap[0][0])
            for t in range(T):
                if t % RANGE == 0 and t > 0:
                    refill(t // RANGE + 2)
                cur = strips[t % SLACK]
                prev = strips[(t - 1) % SLACK]
                slot = (t % RING) * 2 * CW
                d0 = bass.AP(prev.tensor, prev.offset + 2, [[pstr, P], [2, CW], [-2, 2]])
                eng.add_instruction(
                    mybir.InstTensorScalarPtr(
                        name=nc.get_next_instruction_name(),
                        is_tensor_tensor_scan=True,
                        is_scalar_tensor_tensor=True,
                        op0=mybir.AluOpType.min,
                        op1=mybir.AluOpType.add,
                        ins=[
                            eng.lower_ap(d0),
                            eng.lower_ap(cur[:, 0:1]),
                            eng.lower_ap(t_ring[:, slot : slot + 2 * CW]),
                        ],
                        outs=[eng.lower_ap(cur[:, 1 : 2 * CW + 1])],
                    )
                )
                if t == 0:
                    # the 0.0 corner must be INF for every later read
                    nc.vector.memset(strips[SLACK - 1][0:1, 0:1], float(INF))
                if t >= 1:
                    # boundary for scan(t-1): shift lastcol of its strip to
                    # partition p+1 (PE matmul -> PSUM), add the [INF,0,..]
                    # bias on the scalar engine (only engine besides DVE that
                    # reads PSUM), store into the same strip's halo slot,
                    # which scan(t-1+SLACK) reads as its initial value and
                    # scan(t+SLACK) reads as the upleft d0 element.
                    pcur = strips[(t - 1) % SLACK]
                    ps = psp.tile([P, 1], F32, tag="ps", name=f"ps{t}")
                    nc.tensor.matmul(ps[:], t_shift[:], pcur[:, 2 * CW : 2 * CW + 1])
                    nc.scalar.activation(
                        pcur[:, 0:1],
                        ps[:],
                        mybir.ActivationFunctionType.Identity,
                        bias=t_bias[:, 0:1],
                        scale=1.0,
                    )

            final = strips[(T - 1) % SLACK]
            nc.sync.dma_start(res[0:1, 0:1], final[P - 1 : P, 2 * CW : 2 * CW + 1])
            if debug_dump:
                nc.sync.dma_start(dbg_sdiag[:], t_sdiag[:])
                nc.sync.dma_start(dbg_shift[:], t_shift[:])
                nc.sync.dma_start(dbg_bias[:], t_bias[:])
    nc.compile()
    return nc


def _prep_inputs(source, target):
    source = np.ascontiguousarray(np.asarray(source, np.float32).reshape(1, N))
    negt = np.ascontiguousarray(-np.asarray(target, np.float32).reshape(P, CW))
    return {"src": source, "negt": negt}


def _make_runner(nc):
    """Process-cached single-jit dispatch of nc on 8 cores (the computation
    run_bass_via_pjrt builds, without its per-call retrace/recompile)."""
    import jax
    from jax.experimental.shard_map import shard_map
    from jax.sharding import Mesh, PartitionSpec

    from concourse.bass2jax import (
        _bass_exec_p,
        install_neuronx_cc_hook,
        partition_id_tensor,
    )

    install_neuronx_cc_hook()

    partition_name = nc.partition_id_tensor.name if nc.partition_id_tensor else None
    in_names, out_names, out_avals, zero_shapes = [], [], [], []
    for alloc in nc.m.functions[0].allocations:
        if not isinstance(alloc, mybir.MemoryLocationSet):
            continue
        name = alloc.memorylocations[0].name
        if alloc.kind == "ExternalInput":
            if name != partition_name:
                in_names.append(name)
        elif alloc.kind == "ExternalOutput":
            shape = tuple(alloc.tensor_shape)
            dtype = mybir.dt.np(alloc.dtype)
            out_avals.append(jax.core.ShapedArray(shape, dtype))
            zero_shapes.append((shape, dtype))
            out_names.append(name)
    n_params = len(in_names)
    n_outs = len(out_avals)
    in_names_full = list(in_names) + out_names
    if partition_name is not None:
        in_names_full.append(partition_name)

    def _body(*args):
        operands = list(args)
        if partition_name is not None:
            operands.append(partition_id_tensor())
        outs = _bass_exec_p.bind(
            *operands,
            out_avals=tuple(out_avals),
            in_names=tuple(in_names_full),
            out_names=tuple(out_names),
            lowering_input_output_aliases=(),
            sim_require_finite=True,
            sim_require_nnan=True,
            nc=nc,
        )
        return tuple(outs)

    devices = jax.devices()[:NCORES]
    mesh = Mesh(np.asarray(devices), ("core",))
    jf = jax.jit(
        shard_map(
            _body,
            mesh=mesh,
            in_specs=(PartitionSpec("core"),) * (n_params + n_outs),
            out_specs=(PartitionSpec("core"),) * n_outs,
            check_rep=False,
        ),
        donate_argnums=tuple(range(n_params, n_params + n_outs)),
        keep_unused=True,
    )

    def run(in_map):
        concat_in = [
            np.concatenate([np.asarray(in_map[name])] * NCORES, axis=0)
            for name in in_names
        ]
        concat_zeros = [
            np.zeros((NCORES * s[0], *s[1:]), d) for s, d in zero_shapes
        ]
        out = jf(*concat_in, *concat_zeros)
        arr = np.asarray(out[out_names.index("res")])
        return arr.reshape(NCORES, 1, 1)[0]

    return run


def kernel(source, target):
    inputs = _prep_inputs(source, target)
    if "runner" not in _cache:
        nc = _build()
        _cache["nc"] = nc
        r = run_bass_kernel_spmd(
            nc, [dict(inputs) for _ in range(NCORES)], core_ids=list(range(NCORES))
        )
        loss_sq = r.results[0]["res"][0, 0]
        runner = _make_runner(nc)
        runner(inputs)  # warm the cached jit (wrapper XLA compile)
        _cache["runner"] = runner
    else:
        loss_sq = _cache["runner"](inputs)[0, 0]
    return np.sqrt(np.float32(loss_sq))[None].astype(np.float32)


# revision 22
# speedup vs baseline: 1.7799x; 1.7799x over previous
"""DTW loss kernel for Trainium2 (Bass).

Computes sqrt(DTW^2(source, target)) for source, target of shape (2048,)
via the standard DP:
    D[i,j] = (s_i - t_j)^2 + min(D[i-1,j], D[i,j-1], D[i-1,j-1])

Device mapping (one NeuronCore; the problem is a single (source,target)
pair, so all 8 cores run the same program replicated and core 0's output
is used):

- 128 column-chunks of 16 columns each; partition p owns columns [16p,16p+16).
- Wavefront: at step t partition p computes DP row r = t - 2*p (2 steps of
  slack per chunk so the boundary machinery stays off the critical path).
- One DP row-chunk = ONE vector-engine tensor_tensor_scan instruction:
  state = min(d0, state) + d1 over 32 interleaved slots (2 per cell):
    slot 2j   : d0 = D[r-1, j]   (up),      d1 = 0
    slot 2j+1 : d0 = D[r-1, j-1] (upleft),  d1 = c[r, j]
  so after slot 2j+1: state = c + min(up, upleft, left-carry)  == D[r, j].
  d0 is a 3-D access pattern (positions 2+2j-2i) over the previous row strip.
- Cross-chunk boundary (D[r, 16p-1]): PE matmul with a shifted-identity matrix
  moves each chunk's last column value to partition p+1 (PSUM), then the
  scalar engine copies it into the next strip's halo slot, adding a
  per-partition bias [1e30, 0, ...] to restore partition 0's boundary = INF.
- Costs c[r,j] are produced in bulk by the scalar engine (Square activation
  with per-partition bias -t_j) into a ring buffer from a diagonally-shifted
  copy of source.

Host<->device traffic is kept minimal: only source (8KB) and -target (8KB)
are shipped per core; the diagonally-shifted source copy, the shift matrix
and the boundary bias are constructed on device (memsets + two DMAs, one of
them a stride-0 partition broadcast with a skewed output access pattern).

Dispatch: run_bass_kernel_spmd re-traces and re-compiles a fresh
jax.jit(shard_map(...)) on every call (~0.9s each under axon). The first
kernel() call goes through it once; subsequent calls reuse a process-cached
jit of the identical _bass_exec_p computation, so a steady-state call costs
one axon round trip.
"""

import os
import sys

for _p in ("/opt/trn_rl_repo", "/root/.axon_site/_ro/trn_rl_repo"):
    if os.path.isdir(_p) and _p not in sys.path:
        sys.path.insert(0, _p)

import numpy as np

import concourse.bass as bass
import concourse.bacc as bacc
import concourse.mybir as mybir
import concourse.tile as tile
from concourse.bass_utils import run_bass_kernel_spmd

F32 = mybir.dt.float32

N = 2048            # sequence length (both source and target)
P = 128             # partitions / column chunks
CW = N // P         # columns per chunk
SW = 2 * CW + 2     # strip width: [halo | 2*CW scan slots | pad]
SLACK = 4           # wavefront steps of slack per chunk (= number of strips)
T = N + SLACK * (P - 1)   # 2302 total wavefront steps
M = T + 2                 # sdiag columns (padded)
RANGE = 256               # ring refill granularity (steps)
NRANGE = (T + RANGE - 1) // RANGE
RING = 3 * RANGE          # ring capacity in row-slots
INF = np.float32(1e30)
PAD = np.float32(1e15)    # sdiag pad; squares to 1e30
NCORES = 8

_cache = {}


def _build(debug_dump=False):
    nc = bacc.Bacc("TRN2", target_bir_lowering=False, debug=False)

    src = nc.dram_tensor("src", [1, N], F32, kind="ExternalInput")
    negt = nc.dram_tensor("negt", [P, CW], F32, kind="ExternalInput")
    res = nc.dram_tensor("res", [1, 1], F32, kind="ExternalOutput")
    if debug_dump:
        dbg_sdiag = nc.dram_tensor("dbg_sdiag", [P, M], F32, kind="ExternalOutput")
        dbg_shift = nc.dram_tensor("dbg_shift", [P, P], F32, kind="ExternalOutput")
        dbg_bias = nc.dram_tensor("dbg_bias", [P, 1], F32, kind="ExternalOutput")

    with tile.TileContext(nc) as tc:
        with (
            tc.tile_pool(name="sb", bufs=1) as pool,
            tc.tile_pool(name="ps", bufs=8, space="PSUM") as psp,
        ):
            t_sdiag = pool.tile([P, M], F32)
            t_negt = pool.tile([P, CW], F32)
            t_shift = pool.tile([P, P], F32)
            t_bias = pool.tile([P, 1], F32)
            t_ring = pool.tile([P, RING * 2 * CW], F32)
            t_tmp = pool.tile([P, RANGE], F32)
            strips = [
                pool.tile([P, SW], F32, name=f"strip{i}") for i in range(SLACK)
            ]

            nc.sync.dma_start(t_negt[:], negt[:])

            # sdiag[p, SLACK*p : SLACK*p + N] = source, PAD elsewhere.
            # Built by log-doubling SBUF->SBUF copies: seed row 0, then each
            # round copies partitions [0, 2^k) to [2^k, 2^{k+1}) at a uniform
            # free offset +SLACK*2^k. Every AP uses the exact partition pitch
            # and constant offsets (skewed partition strides mis-lower in the
            # HW DGE, which restarts accumulation at 4-partition granules).
            nc.vector.memset(t_sdiag[:], float(PAD))
            sstr = int(t_sdiag.ap[0][0])
            nc.sync.dma_start(t_sdiag[0:1, 0:N], src[:])
            for k in range(7):
                blk = 1 << k
                w = N + SLACK * (blk - 1)
                sd_in = bass.AP(t_sdiag.tensor, t_sdiag.offset, [[sstr, blk], [1, w]])
                sd_out = bass.AP(
                    t_sdiag.tensor,
                    t_sdiag.offset + blk * sstr + SLACK * blk,
                    [[sstr, blk], [1, w]],
                )
                nc.sync.dma_start(sd_out, sd_in)

            # shift matrix: ones on the (p-1, p) diagonal, via affine_select
            # (iota = q - p - 1 == 0 keeps the broadcast 1.0, else fill 0).
            # A strided 4-byte SBUF->SBUF DMA scatter writes garbage on HW,
            # and engines can't address single partitions at arbitrary bases.
            nc.gpsimd.memset(t_shift[:], 1.0)
            nc.gpsimd.affine_select(
                t_shift[:],
                t_shift[:],
                pattern=[[1, P]],
                compare_op=mybir.AluOpType.is_equal,
                fill=0.0,
                base=-1,
                channel_multiplier=-1,
            )

            # boundary bias: [INF, 0, 0, ...]
            nc.vector.memset(t_bias[:], 0.0)
            nc.vector.memset(t_bias[0:1, 0:1], float(INF))

            # zeros in the even (d1) slots persist for the whole run
            nc.gpsimd.memset(t_ring[:], 0.0)
            for s in strips:
                nc.vector.memset(s[:], float(INF))
            # corner DTW[0,0] = 0 for the virtual row read by scan(0)
            # (scan(0)'s prev strip is strips[SLACK-1])
            nc.vector.memset(strips[SLACK - 1][0:1, 0:1], 0.0)

            eng = nc.vector

            def refill(g):
                # Costs are produced on the Pool engine (add then square) so
                # the scalar engine only runs the per-step boundary copy and
                # refill bursts never queue in front of it.
                if g >= NRANGE:
                    return
                m0 = g * RANGE
                cnt = min(RANGE, M - m0)
                base = (m0 % RING) * 2 * CW
                rstr = int(t_ring.ap[0][0])
                for j in range(CW):
                    out_ap = bass.AP(
                        t_ring.tensor,
                        t_ring.offset + base + 2 * j + 1,
                        [[rstr, P], [2 * CW, cnt]],
                    )
                    nc.gpsimd.tensor_scalar(
                        t_tmp[:, 0:cnt],
                        t_sdiag[:, m0 : m0 + cnt],
                        t_negt[:, j : j + 1],
                        None,
                        mybir.AluOpType.add,
                    )
                    nc.gpsimd.tensor_mul(out_ap, t_tmp[:, 0:cnt], t_tmp[:, 0:cnt])

            refill(0)
            refill(1)
            refill(2)

            pstr = int(strips[0].ap[0][0])
            for t in range(T):
                if t % RANGE == 0 and t > 0:
                    refill(t // RANGE + 2)
                cur = strips[t % SLACK]
                prev = strips[(t - 1) % SLACK]
                slot = (t % RING) * 2 * CW
                d0 = bass.AP(prev.tensor, prev.offset + 2, [[pstr, P], [2, CW], [-2, 2]])
                eng.add_instruction(
                    mybir.InstTensorScalarPtr(
                        name=nc.get_next_instruction_name(),
                        is_tensor_tensor_scan=True,
                        is_scalar_tensor_tensor=True,
                        op0=mybir.AluOpType.min,
                        op1=mybir.AluOpType.add,
                        ins=[
                            eng.lower_ap(d0),
                            eng.lower_ap(cur[:, 0:1]),
                            eng.lower_ap(t_ring[:, slot : slot + 2 * CW]),
                        ],
                        outs=[eng.lower_ap(cur[:, 1 : 2 * CW + 1])],
                    )
                )
                if t == 0:
                    # the 0.0 corner must be INF for every later read
                    nc.vector.memset(strips[SLACK - 1][0:1, 0:1], float(INF))
                if t >= 1:
                    # boundary for scan(t-1): shift lastcol of its strip to
                    # partition p+1 (PE matmul -> PSUM), add the [INF,0,..]
                    # bias on the scalar engine (only engine besides DVE that
                    # reads PSUM), store into the same strip's halo slot,
                    # which scan(t-1+SLACK) reads as its initial value and
                    # scan(t+SLACK) reads as the upleft d0 element.
                    pcur = strips[(t - 1) % SLACK]
                    ps = psp.tile([P, 1], F32, tag="ps", name=f"ps{t}")
                    nc.tensor.matmul(ps[:], t_shift[:], pcur[:, 2 * CW : 2 * CW + 1])
                    nc.scalar.activation(
                        pcur[:, 0:1],
                        ps[:],
                        mybir.ActivationFunctionType.Identity,
                        bias=t_bias[:, 0:1],
                        scale=1.0,
                    )

            final = strips[(T - 1) % SLACK]
            nc.sync.dma_start(res[0:1, 0:1], final[P - 1 : P, 2 * CW : 2 * CW + 1])
            if debug_dump:
                nc.sync.dma_start(dbg_sdiag[:], t_sdiag[:])
                nc.sync.dma_start(dbg_shift[:], t_shift[:])
                nc.sync.dma_start(dbg_bias[:], t_bias[:])
    nc.compile()
    return nc


def _prep_inputs(source, target):
    source = np.ascontiguousarray(np.asarray(source, np.float32).reshape(1, N))
    negt = np.ascontiguousarray(-np.asarray(target, np.float32).reshape(P, CW))
    return {"src": source, "negt": negt}


def _make_runner(nc):
    """Process-cached single-jit dispatch of nc on 8 cores (the computation
    run_bass_via_pjrt builds, without its per-call retrace/recompile)."""
    import jax
    from jax.experimental.shard_map import shard_map
    from jax.sharding import Mesh, PartitionSpec

    from concourse.bass2jax import (
        _bass_exec_p,
        install_neuronx_cc_hook,
        partition_id_tensor,
    )

    install_neuronx_cc_hook()

    partition_name = nc.partition_id_tensor.name if nc.partition_id_tensor else None
    in_names, out_names, out_avals, zero_shapes = [], [], [], []
    for alloc in nc.m.functions[0].allocations:
        if not isinstance(alloc, mybir.MemoryLocationSet):
            continue
        name = alloc.memorylocations[0].name
        if alloc.kind == "ExternalInput":
            if name != partition_name:
                in_names.append(name)
        elif alloc.kind == "ExternalOutput":
            shape = tuple(alloc.tensor_shape)
            dtype = mybir.dt.np(alloc.dtype)
            out_avals.append(jax.core.ShapedArray(shape, dtype))
            zero_shapes.append((shape, dtype))
            out_names.append(name)
    n_params = len(in_names)
    n_outs = len(out_avals)
    in_names_full = list(in_names) + out_names
    if partition_name is not None:
        in_names_full.append(partition_name)

    def _body(*args):
        operands = list(args)
        if partition_name is not None:
            operands.append(partition_id_tensor())
        outs = _bass_exec_p.bind(
            *operands,
            out_avals=tuple(out_avals),
            in_names=tuple(in_names_full),
            out_names=tuple(out_names),
            lowering_input_output_aliases=(),
            sim_require_finite=True,
            sim_require_nnan=True,
            nc=nc,
        )
        return tuple(outs)

    devices = jax.devices()[:NCORES]
    mesh = Mesh(np.asarray(devices), ("core",))
    jf = jax.jit(
        shard_map(
            _body,
            mesh=mesh,
            in_specs=(PartitionSpec("core"),) * (n_params + n_outs),
            out_specs=(PartitionSpec("core"),) * n_outs,
            check_rep=False,
        ),
        donate_argnums=tuple(range(n_params, n_params + n_outs)),
        keep_unused=True,
    )

    def run(in_map):
        concat_in = [
            np.concatenate([np.asarray(in_map[name])] * NCORES, axis=0)
            for name in in_names
        ]
        concat_zeros = [
            np.zeros((NCORES * s[0], *s[1:]), d) for s, d in zero_shapes
        ]
        out = jf(*concat_in, *concat_zeros)
        arr = np.asarray(out[out_names.index("res")])
        return arr.reshape(NCORES, 1, 1)[0]

    return run


def kernel(source, target):
    inputs = _prep_inputs(source, target)
    if "runner" not in _cache:
        nc = _build()
        _cache["nc"] = nc
        runner = _make_runner(nc)
        loss_sq = None
        try:
            r = run_bass_kernel_spmd(
                nc,
                [dict(inputs) for _ in range(NCORES)],
                core_ids=list(range(NCORES)),
            )
            loss_sq = r.results[0]["res"][0, 0]
        except Exception:
            pass  # e.g. BASS_TRACE set with no NTFF hook; cached jit below
        warm = runner(inputs)  # warm the cached jit (wrapper XLA compile)
        if loss_sq is None:
            loss_sq = warm[0, 0]
        _cache["runner"] = runner
    else:
        loss_sq = _cache["runner"](inputs)[0, 0]
    return np.sqrt(np.float32(loss_sq))[None].astype(np.float32)
